# revision 1
# baseline (speedup 1.0000x reference)
"""Trainium2 Bass kernel for nn_ConvG (3-level GCN + TopK pooling + readout).

Strategy: data-parallel over the 8 NeuronCores (16 graphs each). On the host,
the edge list is converted to dense per-graph adjacency count matrices
A[g][s][d] = #edges(s->d) + I (a pure format conversion; the initial edge
mask is all-ones so this is data-independent). On device, everything runs in
the original 256-node index space with a cumulative keep-mask kv per node:

  prop:   deg[d] = sum_s kv[s] * A[s,d]  (= 1 + masked in-degree for kept d)
          dinv = 1/sqrt(deg);  2 hops of h <- (dinv*kv) o (A^T @ ((dinv*kv) o h))
          with an intermediate (dinv^2*kv) scale between hops
  pool:   score = h @ (pw/||pw||); top-k among active rows via max8/match-
          replace; h <- h * kv_new * tanh(score)
  readout: masked max (additive -1e30 at dropped nodes) and sum/k over nodes

This reproduces the reference exactly up to fp32 rounding (validated against
the JAX reference in numpy at ~1e-5 rel err).
"""
import numpy as np

G = 16            # graphs per core
N = 256           # nodes per graph
F_IN = 128
H1 = 256
H2 = 128
C = 10
NCORES = 8
B = G * NCORES    # 128 graphs
KS = [205, 164, 132]
BIG = 1e30
MINV = -1e30

_CACHE = {}


def _build():
    import concourse.bacc as bacc
    import concourse.mybir as mybir
    import concourse.tile as tile
    from concourse.masks import make_identity

    dt = mybir.dt.float32
    AF = mybir.ActivationFunctionType
    OP = mybir.AluOpType
    AX = mybir.AxisListType

    nc = bacc.Bacc("TRN2", target_bir_lowering=False, debug=False)

    x_d = nc.dram_tensor("x", [G * N, F_IN], dt, kind="ExternalInput")
    adj_d = nc.dram_tensor("adj", [G, N, N], dt, kind="ExternalInput")
    w12_d = nc.dram_tensor("w12", [F_IN, H1], dt, kind="ExternalInput")
    w22_d = nc.dram_tensor("w22", [H1, H1], dt, kind="ExternalInput")
    w32_d = nc.dram_tensor("w32", [H1, H1], dt, kind="ExternalInput")
    w1_d = nc.dram_tensor("w1", [2 * H1, H1], dt, kind="ExternalInput")
    w2_d = nc.dram_tensor("w2", [H1, H2], dt, kind="ExternalInput")
    w3_d = nc.dram_tensor("w3", [H2, C], dt, kind="ExternalInput")
    b12_d = nc.dram_tensor("b12", [1, H1], dt, kind="ExternalInput")
    b22_d = nc.dram_tensor("b22", [1, H1], dt, kind="ExternalInput")
    b32_d = nc.dram_tensor("b32", [1, H1], dt, kind="ExternalInput")
    b1_d = nc.dram_tensor("b1", [1, H1], dt, kind="ExternalInput")
    b2_d = nc.dram_tensor("b2", [1, H2], dt, kind="ExternalInput")
    b3_d = nc.dram_tensor("b3", [1, C], dt, kind="ExternalInput")
    pwb_d = [nc.dram_tensor(f"pwb{i}", [128, H1], dt, kind="ExternalInput")
             for i in range(3)]
    out_d = nc.dram_tensor("out", [G, C], dt, kind="ExternalOutput")

    GN = G * N  # 4096

    with tile.TileContext(nc) as tc:
        import contextlib
        with contextlib.ExitStack() as ctx:
            big = ctx.enter_context(tc.tile_pool(name="big", bufs=1))
            sm = ctx.enter_context(tc.tile_pool(name="sm", bufs=1))
            sq_pool = ctx.enter_context(tc.tile_pool(name="sqp", bufs=3))
            pmm = ctx.enter_context(tc.tile_pool(name="pmm", bufs=4, space="PSUM"))
            pt = ctx.enter_context(tc.tile_pool(name="pt", bufs=3, space="PSUM"))
            pdeg = ctx.enter_context(tc.tile_pool(name="pdeg", bufs=1, space="PSUM"))

            HN = big.tile([128, 2 * GN], dt, tag="HN")    # node-major h
            U = big.tile([128, 2 * GN], dt, tag="U")      # scratch
            HT = [big.tile([128, GN], dt, tag=f"HT{i}", name=f"HT{i}") for i in range(2)]
            ADJ = big.tile([128, 2 * GN], dt, tag="ADJ")

            W12S = sm.tile([128, H1], dt, tag="w12s")
            W22S = sm.tile([128, 2 * H1], dt, tag="w22s")
            W32S = sm.tile([128, 2 * H1], dt, tag="w32s")
            W1S = sm.tile([128, 4 * H1], dt, tag="w1s")
            W2S = sm.tile([128, 2 * H2], dt, tag="w2s")
            W3S = sm.tile([128, C], dt, tag="w3s")
            B12R = sm.tile([1, H1], dt, tag="b12r")
            B22R = sm.tile([1, H1], dt, tag="b22r")
            B32R = sm.tile([1, H1], dt, tag="b32r")
            B1R = sm.tile([1, H1], dt, tag="b1r")
            B2R = sm.tile([1, H2], dt, tag="b2r")
            B3R = sm.tile([1, C], dt, tag="b3r")
            PWB = [sm.tile([128, H1], dt, tag=f"pwb{i}", name=f"PWB{i}") for i in range(3)]

            IDT = sm.tile([128, 128], dt, tag="idt")
            ONESR = sm.tile([1, 128], dt, tag="onesr")
            EPSB = sm.tile([128, 1], dt, tag="epsb")

            KV = sm.tile([16, N], dt, tag="kv")
            KVT = [sm.tile([128, G], dt, tag=f"kvt{i}", name=f"KVT{i}") for i in range(2)]
            DICB = sm.tile([128, 2 * G], dt, tag="dicb")
            KD = [sm.tile([128, G], dt, tag=f"kd{i}", name=f"KD{i}") for i in range(2)]
            KD2 = [sm.tile([128, G], dt, tag=f"kd2{i}", name=f"KD2{i}") for i in range(2)]
            SCC = [sm.tile([128, G], dt, tag=f"scc{i}", name=f"SCC{i}") for i in range(2)]
            PST = [sm.tile([128, G], dt, tag=f"pst{i}", name=f"PST{i}") for i in range(2)]
            SC = sm.tile([16, N], dt, tag="sc")
            SCM = sm.tile([16, N], dt, tag="scm")
            AM16 = sm.tile([16, N], dt, tag="am16")
            WRK = sm.tile([16, N], dt, tag="wrk")
            MSK = sm.tile([16, N], dt, tag="msk")
            TH = sm.tile([16, N], dt, tag="th")
            PS = sm.tile([16, N], dt, tag="ps")
            TK8 = sm.tile([16, 8], dt, tag="tk8")

            ZACC = sm.tile([128, 64], dt, tag="zacc")
            ZTL = sm.tile([128, 64], dt, tag="ztl")
            Z1 = sm.tile([16, H1], dt, tag="z1")
            Z1T = sm.tile([128, 2 * G], dt, tag="z1t")
            Z2 = sm.tile([16, H2], dt, tag="z2")
            Z2T = sm.tile([128, G], dt, tag="z2t")
            M16 = sm.tile([16, 1], dt, tag="m16")
            NM16 = sm.tile([16, 1], dt, tag="nm16")
            ES = sm.tile([16, C], dt, tag="es")
            SE = sm.tile([16, 1], dt, tag="se")
            LSE = sm.tile([16, 1], dt, tag="lse")
            OUTS = sm.tile([16, C], dt, tag="outs")

            def hsl(g, t):  # HN/U/T1 slice for (graph, node-half)
                o = (g * 2 + t) * N
                return slice(o, o + N)

            # ---- consts + input DMAs
            make_identity(nc, IDT[:])
            nc.gpsimd.memset(ONESR[:], 1.0)
            nc.gpsimd.memset(EPSB[:], 1e-12)
            nc.gpsimd.memset(KV[:], 1.0)
            nc.gpsimd.memset(KVT[0][:], 1.0)
            nc.gpsimd.memset(KVT[1][:], 1.0)
            nc.gpsimd.memset(ZACC[:], 0.0)

            nc.sync.dma_start(U[:, 0:GN].rearrange("p (t f) -> p t f", t=32),
                  x_d[:].rearrange("(t p) f -> p t f", p=128))
            nc.sync.dma_start(ADJ[:].rearrange("p (g t d) -> p g t d", g=G, t=2),
                  adj_d[:].rearrange("g (t p) d -> p g t d", p=128))
            nc.sync.dma_start(W12S[:], w12_d[:])
            nc.sync.dma_start(W22S[:].rearrange("p (t n) -> p t n", n={"W22S":256,"W32S":256,"W1S":256,"W2S":128}["W22S"]),
                  w22_d[:].rearrange("(t p) n -> p t n", p=128))
            nc.sync.dma_start(W32S[:].rearrange("p (t n) -> p t n", n={"W22S":256,"W32S":256,"W1S":256,"W2S":128}["W32S"]),
                  w32_d[:].rearrange("(t p) n -> p t n", p=128))
            nc.sync.dma_start(W1S[:].rearrange("p (t n) -> p t n", n={"W22S":256,"W32S":256,"W1S":256,"W2S":128}["W1S"]),
                  w1_d[:].rearrange("(t p) n -> p t n", p=128))
            nc.sync.dma_start(W2S[:].rearrange("p (t n) -> p t n", n={"W22S":256,"W32S":256,"W1S":256,"W2S":128}["W2S"]),
                  w2_d[:].rearrange("(t p) n -> p t n", p=128))
            nc.sync.dma_start(W3S[:], w3_d[:])
            for dst, src in ((B12R, b12_d), (B22R, b22_d), (B32R, b32_d),
                             (B1R, b1_d), (B2R, b2_d), (B3R, b3_d)):
                nc.sync.dma_start(dst[:], src[:])
            for i in range(3):
                nc.sync.dma_start(PWB[i][:], pwb_d[i][:])

            # ---- xT into HT0 (level-1 feature-major input; F_IN = 128)
            for i in range(32):
                pp = pt.tile([128, 128], dt, tag="psT")
                nc.tensor.transpose(pp[:], U[:, i * 128:(i + 1) * 128], IDT[:])
                if i % 2 == 0:
                    nc.scalar.copy(HT[0][:, i * 128:(i + 1) * 128], pp[:])
                else:
                    nc.vector.tensor_copy(HT[0][:, i * 128:(i + 1) * 128], pp[:])

            def dense(lvl):
                """HT (feature-major) -> HN = relu(h @ W + b), node-major."""
                WS, BR, kts = {1: (W12S, B12R, 1), 2: (W22S, B22R, 2),
                               3: (W32S, B32R, 2)}[lvl]
                for g in range(G):
                    for mt in range(2):
                        ps = pmm.tile([128, H1], dt, tag="psA")
                        o = g * N + mt * 128
                        for kt in range(kts):
                            nc.tensor.matmul(
                                ps[:], HT[kt][:, o:o + 128],
                                WS[:, kt * H1:(kt + 1) * H1],
                                start=(kt == 0), stop=False)
                        nc.tensor.matmul(ps[:], ONESR[0:1, 0:128], BR[:],
                                         start=False, stop=True)
                        nc.scalar.activation(HN[:, hsl(g, mt)], ps[:], AF.Relu)

            def prop():
                # degrees -> dinv columns -> kd = dinv*kv, kd2 = dinv^2*kv
                pdg = pdeg.tile([128, 2 * G], dt, tag="psD")
                for g in range(G):
                    for dh in range(2):
                        col = dh * G + g
                        for st in range(2):
                            ao = (g * 2 + st) * N + dh * 128
                            nc.tensor.matmul(pdg[:, col:col + 1],
                                             ADJ[:, ao:ao + 128],
                                             KVT[st][:, g:g + 1],
                                             start=(st == 0), stop=(st == 1))
                sqc = sq_pool.tile([128, 2 * G], dt, tag="sq")
                nc.scalar.activation(sqc[:], pdg[:], AF.Sqrt, bias=EPSB[:, 0:1])
                nc.vector.reciprocal(DICB[:], sqc[:])
                for dh in range(2):
                    nc.vector.tensor_mul(KD[dh][:], DICB[:, dh * G:(dh + 1) * G],
                                         KVT[dh][:])
                    nc.vector.tensor_mul(KD2[dh][:], KD[dh][:],
                                         DICB[:, dh * G:(dh + 1) * G])
                # u = kd o h
                for g in range(G):
                    for t in range(2):
                        nc.vector.tensor_scalar_mul(U[:, hsl(g, t)],
                                                    HN[:, hsl(g, t)],
                                                    KD[t][:, g:g + 1])
                # hop 1: u <- kd2 o (A^T @ u)   (in place, via two psums)
                for g in range(G):
                    pss = []
                    for dh in range(2):
                        ps = pmm.tile([128, H1], dt, tag="psA")
                        for st in range(2):
                            ao = (g * 2 + st) * N + dh * 128
                            nc.tensor.matmul(ps[:], ADJ[:, ao:ao + 128],
                                             U[:, hsl(g, st)],
                                             start=(st == 0), stop=(st == 1))
                        pss.append(ps)
                    for dh in range(2):
                        nc.vector.tensor_scalar_mul(U[:, hsl(g, dh)], pss[dh][:],
                                                    KD2[dh][:, g:g + 1])
                # hop 2: h = kd o (A^T @ u)
                for g in range(G):
                    for dh in range(2):
                        ps = pmm.tile([128, H1], dt, tag="psA")
                        for st in range(2):
                            ao = (g * 2 + st) * N + dh * 128
                            nc.tensor.matmul(ps[:], ADJ[:, ao:ao + 128],
                                             U[:, hsl(g, st)],
                                             start=(st == 0), stop=(st == 1))
                        nc.vector.tensor_scalar_mul(HN[:, hsl(g, dh)], ps[:],
                                                    KD[dh][:, g:g + 1])

            def pool_readout(lvl):
                k = KS[lvl]
                # scores (columns), via fused mul+reduce on DVE
                for g in range(G):
                    for mt in range(2):
                        nc.vector.tensor_mul(U[:, hsl(g, mt)],
                                             HN[:, hsl(g, mt)], PWB[lvl][:])
                        nc.vector.tensor_reduce(SCC[mt][:, g:g + 1],
                                                U[:, hsl(g, mt)],
                                                axis=AX.X, op=OP.add)
                # score rows [16, 256]
                for mt in range(2):
                    pp = pt.tile([128, 128], dt, tag="psT")
                    nc.tensor.transpose(pp[0:16, :], SCC[mt][:], IDT[:])
                    nc.scalar.copy(SC[:, mt * 128:(mt + 1) * 128], pp[0:16, :])
                # mask inactive scores to -BIG
                nc.vector.tensor_scalar(AM16[:], KV[:], 1.0, BIG,
                                        op0=OP.subtract, op1=OP.mult)
                nc.vector.tensor_add(SCM[:], SC[:], AM16[:])
                # top-k mask via max8 + match_replace
                cur = SCM
                for it in range((k + 7) // 8):
                    nc.vector.max(TK8[:], cur[:])
                    rem = k - it * 8
                    if rem < 8:
                        nc.vector.memset(TK8[:, rem:8], MINV)
                    nc.vector.match_replace(WRK[:], TK8[:], cur[:], MINV)
                    cur = WRK
                nc.vector.tensor_sub(MSK[:], SCM[:], WRK[:])
                nc.vector.tensor_scalar_min(MSK[:], MSK[:], 1.0)
                # update keep state; pool scale ps = kv * tanh(score)
                nc.scalar.activation(TH[:], SC[:], AF.Tanh)
                nc.vector.tensor_mul(KV[:], KV[:], MSK[:])
                nc.vector.tensor_mul(PS[:], KV[:], TH[:])
                for mt in range(2):
                    for src, dsts in ((PS, PST), (KV, KVT)):
                        pp = pt.tile([128, 128], dt, tag="psT")
                        nc.tensor.transpose(pp[:, 0:16],
                                            src[0:16, mt * 128:(mt + 1) * 128],
                                            IDT[0:16, 0:16])
                        nc.vector.tensor_copy(dsts[mt][:], pp[:, 0:16])
                # h <- h * ps  (zeroes dropped rows, scales kept by tanh)
                for g in range(G):
                    for t in range(2):
                        nc.vector.tensor_scalar_mul(HN[:, hsl(g, t)],
                                                    HN[:, hsl(g, t)],
                                                    PST[t][:, g:g + 1])
                # transpose to feature-major HT
                i = 0
                for g in range(G):
                    for mt in range(2):
                        for ft in range(2):
                            pp = pt.tile([128, 128], dt, tag="psT")
                            o = (g * 2 + mt) * N + ft * 128
                            nc.tensor.transpose(pp[:], HN[:, o:o + 128], IDT[:])
                            dst = HT[ft][:, g * N + mt * 128:
                                          g * N + mt * 128 + 128]
                            if i % 2 == 0:
                                nc.scalar.copy(dst, pp[:])
                            else:
                                nc.vector.tensor_copy(dst, pp[:])
                            i += 1
                # readout: additive mask AMB = (kv-1)*BIG broadcast over parts
                nc.sync.dma_start(U[0:1, 0:GN], KV[:])
                for c in range(8):
                    pb = pmm.tile([128, 512], dt, tag="psA")
                    nc.tensor.matmul(pb[:], ONESR[0:1, :],
                                     U[0:1, c * 512:(c + 1) * 512],
                                     start=True, stop=True)
                    nc.vector.tensor_scalar(U[:, GN + c * 512:GN + (c + 1) * 512],
                                            pb[:], 1.0, BIG, op0=OP.subtract,
                                            op1=OP.mult)
                for ft in range(2):
                    nc.vector.tensor_add(U[:, 0:GN], HT[ft][:], U[:, GN:2 * GN])
                    nc.vector.tensor_reduce(
                        ZTL[:, ft * 16:(ft + 1) * 16],
                        U[:, 0:GN].rearrange("p (g n) -> p g n", g=G),
                        axis=AX.X, op=OP.max)
                    nc.vector.tensor_reduce(
                        ZTL[:, (2 + ft) * 16:(3 + ft) * 16],
                        HT[ft][:].rearrange("p (g n) -> p g n", g=G),
                        axis=AX.X, op=OP.add)
                nc.vector.tensor_scalar_mul(ZTL[:, 32:64], ZTL[:, 32:64],
                                            1.0 / k)
                nc.vector.tensor_add(ZACC[:], ZACC[:], ZTL[:])

            # ---- the network
            dense(1)
            for lvl in range(3):
                prop()
                pool_readout(lvl)
                if lvl < 2:
                    dense(lvl + 2)

            # ---- final MLP + log_softmax
            ps1 = pmm.tile([128, H1], dt, tag="psA")
            for kt in range(4):
                nc.tensor.matmul(ps1[0:16, :], ZACC[:, kt * 16:(kt + 1) * 16],
                                 W1S[:, kt * H1:(kt + 1) * H1],
                                 start=(kt == 0), stop=False)
            nc.tensor.matmul(ps1[0:16, :], ONESR[0:1, 0:16], B1R[:],
                             start=False, stop=True)
            nc.scalar.activation(Z1[:], ps1[0:16, :], AF.Relu)
            for kt in range(2):
                pp = pt.tile([128, 128], dt, tag="psT")
                nc.tensor.transpose(pp[:, 0:16],
                                    Z1[0:16, kt * 128:(kt + 1) * 128],
                                    IDT[0:16, 0:16])
                nc.scalar.copy(Z1T[:, kt * 16:(kt + 1) * 16], pp[:, 0:16])
            ps2 = pmm.tile([128, H2], dt, tag="psA")
            for kt in range(2):
                nc.tensor.matmul(ps2[0:16, :], Z1T[:, kt * 16:(kt + 1) * 16],
                                 W2S[:, kt * H2:(kt + 1) * H2],
                                 start=(kt == 0), stop=False)
            nc.tensor.matmul(ps2[0:16, :], ONESR[0:1, 0:16], B2R[:],
                             start=False, stop=True)
            nc.scalar.activation(Z2[:], ps2[0:16, :], AF.Relu)
            pp = pt.tile([128, 128], dt, tag="psT")
            nc.tensor.transpose(pp[:, 0:16], Z2[0:16, :], IDT[0:16, 0:16])
            nc.scalar.copy(Z2T[:], pp[:, 0:16])
            ps3 = pmm.tile([128, C], dt, tag="psA")
            nc.tensor.matmul(ps3[0:16, :], Z2T[:], W3S[:], start=True,
                             stop=False)
            nc.tensor.matmul(ps3[0:16, :], ONESR[0:1, 0:16], B3R[:],
                             start=False, stop=True)
            nc.vector.tensor_reduce(M16[:], ps3[0:16, :], axis=AX.X, op=OP.max)
            nc.vector.tensor_scalar_mul(NM16[:], M16[:], -1.0)
            nc.scalar.activation(ES[:], ps3[0:16, :], AF.Exp,
                                 bias=NM16[0:16, 0:1], scale=1.0)
            nc.vector.tensor_reduce(SE[:], ES[:], axis=AX.X, op=OP.add)
            nc.scalar.activation(LSE[:], SE[:], AF.Ln)
            nc.vector.tensor_scalar(OUTS[:], ps3[0:16, :], M16[0:16, 0:1],
                                    LSE[0:16, 0:1], op0=OP.subtract,
                                    op1=OP.subtract)
            nc.sync.dma_start(out_d[:], OUTS[:])

    nc.compile()
    return nc


def _get_nc():
    if "nc" not in _CACHE:
        _CACHE["nc"] = _build()
    return _CACHE["nc"]


def _host_prep(inputs):
    """Build per-core input maps (shared weights + per-core x/adj slices)."""
    x = np.ascontiguousarray(np.asarray(inputs["x"], np.float32))
    edges = np.asarray(inputs["edges"], np.int32)
    # dense adjacency counts + self loop: A[g, s, d] = #edges(s->d) + I
    src = edges[..., 0].astype(np.int64)
    dst = edges[..., 1].astype(np.int64)
    gidx = np.arange(B, dtype=np.int64)[:, None]
    flat = (gidx * N * N + src * N + dst).ravel()
    A = np.bincount(flat, minlength=B * N * N).astype(np.float32)
    A = A.reshape(B, N, N)
    A += np.eye(N, dtype=np.float32)[None]

    shared = {}
    for name, key in (("w12", "W12"), ("w22", "W22"), ("w32", "W32"),
                      ("w1", "W1"), ("w2", "W2"), ("w3", "W3")):
        shared[name] = np.ascontiguousarray(np.asarray(inputs[key], np.float32))
    for name, key, n in (("b12", "b12", H1), ("b22", "b22", H1),
                         ("b32", "b32", H1), ("b1", "b1", H1),
                         ("b2", "b2", H2), ("b3", "b3", C)):
        shared[name] = np.asarray(inputs[key], np.float32).reshape(1, n)
    for i, key in enumerate(("pw1", "pw2", "pw3")):
        pw = np.asarray(inputs[key], np.float32)
        pwn = pw / np.linalg.norm(pw)
        shared[f"pwb{i}"] = np.ascontiguousarray(
            np.broadcast_to(pwn[None, :], (128, H1)).astype(np.float32))

    in_maps = []
    for c in range(NCORES):
        m = dict(shared)
        m["x"] = np.ascontiguousarray(x[c * G * N:(c + 1) * G * N])
        m["adj"] = np.ascontiguousarray(A[c * G:(c + 1) * G])
        in_maps.append(m)
    return in_maps


def kernel(**inputs):
    from concourse.bass_utils import run_bass_kernel_spmd
    nc = _get_nc()
    in_maps = _host_prep(inputs)
    r = run_bass_kernel_spmd(nc, in_maps, core_ids=list(range(NCORES)))
    out = np.concatenate([r.results[c]["out"] for c in range(NCORES)], axis=0)
    return out.astype(np.float32)


def run_traced(inputs):
    """Like kernel() but with NTFF tracing; returns (out, BassKernelResults)."""
    import sys
    import types
    if "antenv.axon_hooks" not in sys.modules:
        hooks = types.ModuleType("antenv.axon_hooks")
        hooks._hook = None
        hooks.set_axon_ntff_profile_hook = lambda h: setattr(hooks, "_hook", h)
        hooks.get_axon_ntff_profile_hook = lambda: hooks._hook
        sys.modules["antenv.axon_hooks"] = hooks
        from trn_agent_boot.trn_boot import _ntff_profile_via_ctypes
        hooks.set_axon_ntff_profile_hook(
            _ntff_profile_via_ctypes("/opt/axon/libaxon_pjrt.so"))
    from concourse.bass_utils import run_bass_kernel_spmd
    nc = _get_nc()
    in_maps = _host_prep(inputs)
    r = run_bass_kernel_spmd(nc, in_maps, core_ids=list(range(NCORES)),
                             trace=True)
    out = np.concatenate([r.results[c]["out"] for c in range(NCORES)], axis=0)
    return out.astype(np.float32), r



# revision 23
# speedup vs baseline: 2.0789x; 2.0789x over previous
"""Trainium2 Bass kernel for nn_ConvG (3-level GCN + TopK pooling + readout).

Data-parallel over 8 NeuronCores (16 graphs each). Host converts the edge
list to dense per-graph adjacency count matrices A[g][s][d] = #edges(s->d) + I
(bf16) and pre-transposes x (bf16). All matmuls run in bf16 with fp32 PSUM
accumulation (validated vs the fp32 JAX reference at ~1e-4 rel err; gate 2e-2).

No SBUF->SBUF DMAs anywhere (HW lands them late relative to consumers);
row<->column layout changes go through PE transposes, and per-graph row
broadcasts use one-hot selector matmuls (lhsT = E_g -> out[m,n] = row_g[n]).

Per level (kv = cumulative keep mask, kd = dinv*kv, kd2 = dinv^2*kv):
  deg:    column-form matvecs A^T kv -> psum [128, 2G]; sqrt/recip/mask as
          tiny column ops; kd/kd2 columns feed per-partition activation
          scales.
  dense:  u = kd o relu(h @ W): PE (stationary = feature-major h chunks),
          drained by Scalar activation(Relu, scale) / DVE tensor_scalar.
  hop1:   p1 = A^T u (node-major); u2 = kd2 o p1 drains (Scalar/DVE).
  hop2:   p2 = A^T u2 with stationary = u2 chunks -> FEATURE-major psum,
          drained raw to HRAW -- no transposes in the h pipeline.
  score:  s_raw columns via PE (stationary = HRAW chunks, moving = pw col);
          s = s_raw o kd; transposed to rows for top-k.
  topk:   drop-side selection (drop n-k lowest via max8/match_replace of
          negated scores -- 4x fewer iterations than keep-side).
  pool:   cs = kd*kv_new*tanh(s), am = (kv_new-1)*1e4 as columns ->
          transposed to bf16 rows -> selector-matmul broadcast to psum ->
          scalar-copied to SBUF; GpSimd computes h'' = HRAW o cs and
          h''+am (all-SBUF bf16); DVE does grouped readout reduces.
"""
import numpy as np

G = 16            # graphs per core
N = 256           # nodes per graph
F_IN = 128
H1 = 256
H2 = 128
C = 10
NCORES = 8
B = G * NCORES    # 128 graphs
KS = [205, 164, 132]
DROPS = [51, 41, 32]
BIG = 1e30
MINV = -1e30
AMV = 1e4         # additive readout-max mask magnitude

_CACHE = {}


def _build(with_bias):
    import concourse.bacc as bacc
    import concourse.mybir as mybir
    import concourse.tile as tile
    from concourse.masks import make_identity

    f32 = mybir.dt.float32
    bf16 = mybir.dt.bfloat16
    AF = mybir.ActivationFunctionType
    OP = mybir.AluOpType
    AX = mybir.AxisListType

    nc = bacc.Bacc("TRN2", target_bir_lowering=False, debug=False)

    GN = G * N  # 4096

    xt_d = nc.dram_tensor("xt", [F_IN, GN], bf16, kind="ExternalInput")
    adj_d = nc.dram_tensor("adj", [G, N, N], bf16, kind="ExternalInput")
    esel_d = nc.dram_tensor("esel", [G, G * 128], bf16, kind="ExternalInput")
    w12_d = nc.dram_tensor("w12", [F_IN, H1], bf16, kind="ExternalInput")
    w22_d = nc.dram_tensor("w22", [H1, H1], bf16, kind="ExternalInput")
    w32_d = nc.dram_tensor("w32", [H1, H1], bf16, kind="ExternalInput")
    w1_d = nc.dram_tensor("w1", [2 * H1, H1], bf16, kind="ExternalInput")
    w2_d = nc.dram_tensor("w2", [H1, H2], bf16, kind="ExternalInput")
    w3_d = nc.dram_tensor("w3", [H2, C], bf16, kind="ExternalInput")
    pwc_d = nc.dram_tensor("pwc", [128, 6], bf16, kind="ExternalInput")
    b12_d = nc.dram_tensor("b12", [1, H1], bf16, kind="ExternalInput")
    b22_d = nc.dram_tensor("b22", [1, H1], bf16, kind="ExternalInput")
    b32_d = nc.dram_tensor("b32", [1, H1], bf16, kind="ExternalInput")
    b1_d = nc.dram_tensor("b1", [1, H1], bf16, kind="ExternalInput")
    b2_d = nc.dram_tensor("b2", [1, H2], bf16, kind="ExternalInput")
    b3_d = nc.dram_tensor("b3", [1, C], bf16, kind="ExternalInput")
    out_d = nc.dram_tensor("out", [G, C], f32, kind="ExternalOutput")

    with tile.TileContext(nc) as tc:
        import contextlib
        with contextlib.ExitStack() as ctx:
            big = ctx.enter_context(tc.tile_pool(name="big", bufs=1))
            sm = ctx.enter_context(tc.tile_pool(name="sm", bufs=1))
            pmm = ctx.enter_context(tc.tile_pool(name="pmm", bufs=3, space="PSUM"))
            pbc = ctx.enter_context(tc.tile_pool(name="pbc", bufs=2, space="PSUM"))
            pt = ctx.enter_context(tc.tile_pool(name="pt", bufs=1, space="PSUM"))
            pv = ctx.enter_context(tc.tile_pool(name="pv", bufs=2, space="PSUM"))

            ADJ = big.tile([128, 2 * GN], bf16, tag="adj")
            XT = big.tile([128, GN], bf16, tag="xt")
            U = big.tile([128, 2 * GN], bf16, tag="u")
            U2 = big.tile([128, 2 * GN], bf16, tag="u2")
            HRAW = [big.tile([128, GN], bf16, tag=f"hraw{i}", name=f"HRAW{i}")
                    for i in range(2)]
            HM = [big.tile([128, GN], bf16, tag=f"hm{i}", name=f"HM{i}")
                  for i in range(2)]
            HS = [big.tile([128, GN], bf16, tag=f"hs{i}", name=f"HS{i}")
                  for i in range(2)]
            CSSB = big.tile([128, GN], bf16, tag="cssb")
            AMSB = big.tile([128, GN], bf16, tag="amsb")

            ESEL = sm.tile([G, G * 128], bf16, tag="esel")
            W12S = sm.tile([128, H1], bf16, tag="w12s")
            W22S = sm.tile([128, 2 * H1], bf16, tag="w22s")
            W32S = sm.tile([128, 2 * H1], bf16, tag="w32s")
            W1S = sm.tile([128, 4 * H1], bf16, tag="w1s")
            W2S = sm.tile([128, 2 * H2], bf16, tag="w2s")
            W3S = sm.tile([128, C], bf16, tag="w3s")
            PWC = sm.tile([128, 6], bf16, tag="pwc")
            B12R = sm.tile([1, H1], bf16, tag="b12r")
            B22R = sm.tile([1, H1], bf16, tag="b22r")
            B32R = sm.tile([1, H1], bf16, tag="b32r")
            B1R = sm.tile([1, H1], bf16, tag="b1r")
            B2R = sm.tile([1, H2], bf16, tag="b2r")
            B3R = sm.tile([1, C], bf16, tag="b3r")
            BLV = {1: B12R, 2: B22R, 3: B32R}

            IDT = sm.tile([128, 128], f32, tag="idt")
            ONEB = sm.tile([1, 128], bf16, tag="oneb")
            EPSB = sm.tile([128, 1], f32, tag="epsb")

            # column tiles [128, 2G]: col = mt*G + g  (node chunk mt, graph g)
            SQC = sm.tile([128, 2 * G], f32, tag="sqc")
            DICB = sm.tile([128, 2 * G], f32, tag="dicb")
            KVCA = sm.tile([128, 2 * G], f32, tag="kvca")
            KDCA = sm.tile([128, 2 * G], f32, tag="kdca")
            KD2CA = sm.tile([128, 2 * G], f32, tag="kd2ca")
            SCCS = sm.tile([128, 2 * G], f32, tag="sccs")
            SCOL = sm.tile([128, 2 * G], f32, tag="scol")
            THC = sm.tile([128, 2 * G], f32, tag="thc")
            CSC = sm.tile([128, 2 * G], f32, tag="csc")
            AMC = sm.tile([128, 2 * G], f32, tag="amc")
            KVT = [sm.tile([128, G], bf16, tag=f"kvt{i}", name=f"KVT{i}")
                   for i in range(2)]

            # row tiles [16, 256] (one graph per partition)
            KV = sm.tile([16, N], f32, tag="kv")
            KVN = sm.tile([16, N], f32, tag="kvn")
            S = sm.tile([16, N], f32, tag="s")
            AM16 = sm.tile([16, N], f32, tag="am16")
            DS = sm.tile([16, N], f32, tag="ds")
            WRK = sm.tile([16, N], f32, tag="wrk")
            T1 = sm.tile([16, N], f32, tag="t1")
            CSB16 = sm.tile([16, N], bf16, tag="csb16")
            AM3B16 = sm.tile([16, N], bf16, tag="am3b16")
            TK8 = sm.tile([16, 8], f32, tag="tk8")

            # readout accumulators [128, G] per (lvl, kind, ft)
            RDT = [[[sm.tile([128, G], f32, tag=f"rdt{l}{k}{t}",
                             name=f"RDT{l}{k}{t}") for t in range(2)]
                    for k in range(2)] for l in range(3)]
            ZACC = sm.tile([128, 64], f32, tag="zacc")
            SCRC = sm.tile([128, G], f32, tag="scrc")

            Z1 = sm.tile([16, H1], f32, tag="z1")
            Z1T = sm.tile([128, 2 * G], bf16, tag="z1t")
            Z2 = sm.tile([16, H2], f32, tag="z2")
            Z2T = sm.tile([128, G], bf16, tag="z2t")
            M16 = sm.tile([16, 1], f32, tag="m16")
            NM16 = sm.tile([16, 1], f32, tag="nm16")
            ES = sm.tile([16, C], f32, tag="es")
            SE = sm.tile([16, 1], f32, tag="se")
            LSE = sm.tile([16, 1], f32, tag="lse")
            OUTS = sm.tile([16, C], f32, tag="outs")

            def usl(g, t):  # U/U2 column slice for (graph, node-half)
                o = (g * 2 + t) * N
                return slice(o, o + N)

            def asl(g, st):  # ADJ block (graph, src-half): [s128, d256]
                o = (g * 2 + st) * N
                return slice(o, o + N)

            def col(g, mt):
                return slice(mt * G + g, mt * G + g + 1)

            # ---- consts + input DMAs
            make_identity(nc, IDT[:])
            nc.gpsimd.memset(ONEB[:], 1.0)
            nc.gpsimd.memset(EPSB[:], 1e-12)
            nc.gpsimd.memset(KV[:], 1.0)
            nc.gpsimd.memset(KVCA[:], 1.0)
            nc.gpsimd.memset(KVT[0][:], 1.0)
            nc.gpsimd.memset(KVT[1][:], 1.0)

            nc.sync.dma_start(ADJ[:].rearrange("p (g t d) -> p g t d", g=G, t=2),
                              adj_d[:].rearrange("g (t p) d -> p g t d", p=128))
            nc.sync.dma_start(XT[:], xt_d[:])
            nc.sync.dma_start(ESEL[:], esel_d[:])
            nc.sync.dma_start(W12S[:], w12_d[:])
            nc.sync.dma_start(W22S[:].rearrange("p (t n) -> p t n", n=H1),
                              w22_d[:].rearrange("(t p) n -> p t n", p=128))
            nc.sync.dma_start(W32S[:].rearrange("p (t n) -> p t n", n=H1),
                              w32_d[:].rearrange("(t p) n -> p t n", p=128))
            nc.sync.dma_start(W1S[:].rearrange("p (t n) -> p t n", n=H1),
                              w1_d[:].rearrange("(t p) n -> p t n", p=128))
            nc.sync.dma_start(W2S[:].rearrange("p (t n) -> p t n", n=H2),
                              w2_d[:].rearrange("(t p) n -> p t n", p=128))
            nc.sync.dma_start(W3S[:], w3_d[:])
            nc.sync.dma_start(PWC[:], pwc_d[:])
            for dst, src in ((B12R, b12_d), (B22R, b22_d), (B32R, b32_d),
                             (B1R, b1_d), (B2R, b2_d), (B3R, b3_d)):
                nc.sync.dma_start(dst[:], src[:])

            def deg_kd():
                """deg cols = A^T kv -> dinv -> kd/kd2 columns (no DMA)."""
                pdg = pv.tile([128, 2 * G], f32, tag="pcol")
                for g in range(G):
                    for dh in range(2):
                        for st in range(2):
                            ao = (g * 2 + st) * N + dh * 128
                            nc.tensor.matmul(pdg[:, col(g, dh)],
                                             ADJ[:, ao:ao + 128],
                                             KVT[st][:, g:g + 1],
                                             start=(st == 0), stop=(st == 1))
                nc.scalar.activation(SQC[:], pdg[:], AF.Sqrt,
                                     bias=EPSB[:, 0:1])
                nc.vector.reciprocal(DICB[:], SQC[:])
                nc.vector.tensor_mul(KDCA[:], DICB[:], KVCA[:])
                nc.vector.tensor_mul(KD2CA[:], KDCA[:], DICB[:])

            def dense(lvl):
                """U = kd o relu(h @ W), node-major; stationary = h chunks."""
                if lvl == 1:
                    WS, kts = W12S, 1
                else:
                    WS = {2: W22S, 3: W32S}[lvl]
                    kts = 2
                for g in range(G):
                    for mt in range(2):
                        ps = pmm.tile([128, H1], f32, tag="psA")
                        for kt in range(kts):
                            if lvl == 1:
                                lhs = XT[:, g * N + mt * 128:
                                         g * N + mt * 128 + 128]
                            else:
                                lhs = HM[kt][:, g * N + mt * 128:
                                             g * N + mt * 128 + 128]
                            nc.tensor.matmul(ps[:], lhs,
                                             WS[:, kt * H1:(kt + 1) * H1],
                                             start=(kt == 0),
                                             stop=(not with_bias and
                                                   kt == kts - 1))
                        if with_bias:
                            nc.tensor.matmul(ps[:], ONEB[0:1, :], BLV[lvl][:],
                                             start=False, stop=True)
                        dst = U[:, usl(g, mt)]
                        if g % 2 == 0:
                            nc.scalar.activation(dst, ps[:], AF.Relu,
                                                 scale=KDCA[:, col(g, mt)])
                        else:
                            nc.vector.tensor_scalar(dst, ps[:],
                                                    KDCA[:, col(g, mt)], 0.0,
                                                    op0=OP.mult, op1=OP.max)

            def prop():
                # hop1: u2 = kd2 o (A^T u), node-major
                for g in range(G):
                    for dh in range(2):
                        ps = pmm.tile([128, H1], f32, tag="psA")
                        for st in range(2):
                            ao = (g * 2 + st) * N + dh * 128
                            nc.tensor.matmul(ps[:], ADJ[:, ao:ao + 128],
                                             U[:, usl(g, st)],
                                             start=(st == 0), stop=(st == 1))
                        dst = U2[:, usl(g, dh)]
                        if g % 2 == 0:
                            nc.scalar.activation(dst, ps[:], AF.Copy,
                                                 scale=KD2CA[:, col(g, dh)])
                        else:
                            nc.vector.tensor_scalar_mul(dst, ps[:],
                                                        KD2CA[:, col(g, dh)])
                # hop2: p2 = A^T u2, FEATURE-major; drain raw to HRAW
                for g in range(G):
                    for ft in range(2):
                        ps = pmm.tile([128, H1], f32, tag="psA")
                        for st in range(2):
                            uo = (g * 2 + st) * N + ft * 128
                            nc.tensor.matmul(ps[:], U2[:, uo:uo + 128],
                                             ADJ[:, asl(g, st)],
                                             start=(st == 0), stop=(st == 1))
                        dst = HRAW[ft][:, g * N:(g + 1) * N]
                        if g % 2 == 0:
                            nc.scalar.copy(dst, ps[:])
                        else:
                            nc.vector.tensor_copy(dst, ps[:])

            def trow(dst_row, src_col_ap, mt, out_bf=False):
                """[128, G] column-tile slice -> row-tile [16, 128] block."""
                pp = pt.tile([128, 128], f32, tag="pst")
                nc.tensor.transpose(pp[0:16, :], src_col_ap, IDT[:])
                eng = nc.scalar.copy if out_bf else nc.vector.tensor_copy
                eng(dst_row[0:16, mt * 128:(mt + 1) * 128], pp[0:16, :])

            def pool_readout(lvl):
                d = DROPS[lvl]
                # score columns: s_raw = pw . p2 (stationary = HRAW chunks)
                psc = pv.tile([128, 2 * G], f32, tag="pcol")
                for g in range(G):
                    for mt in range(2):
                        for ft in range(2):
                            ho = g * N + mt * 128
                            nc.tensor.matmul(
                                psc[:, col(g, mt)],
                                HRAW[ft][:, ho:ho + 128],
                                PWC[:, lvl * 2 + ft:lvl * 2 + ft + 1],
                                start=(ft == 0), stop=(ft == 1))
                nc.vector.tensor_copy(SCCS[:], psc[:])
                nc.vector.tensor_mul(SCOL[:], SCCS[:], KDCA[:])
                nc.scalar.activation(THC[:], SCOL[:], AF.Tanh)
                for mt in range(2):
                    trow(S, SCOL[:, mt * G:(mt + 1) * G], mt)
                # mask inactive scores; drop-side top-k
                nc.vector.tensor_scalar(AM16[:], KV[:], 1.0, BIG,
                                        op0=OP.subtract, op1=OP.mult)
                nc.vector.tensor_sub(DS[:], AM16[:], S[:])
                cur = DS
                for it in range((d + 7) // 8):
                    nc.vector.max(TK8[:], cur[:])
                    rem = d - it * 8
                    if rem < 8:
                        nc.vector.memset(TK8[:, rem:8], MINV)
                    nc.vector.match_replace(WRK[:], TK8[:], cur[:], MINV)
                    cur = WRK
                # kv_new: 1 where WRK is a kept score (-s), 0 elsewhere
                nc.vector.tensor_scalar(T1[:], WRK[:], 1e-29, 1.0,
                                        op0=OP.mult, op1=OP.add)
                nc.vector.tensor_scalar(KVN[:], T1[:], 0.0, 1.0,
                                        op0=OP.max, op1=OP.min)
                nc.vector.tensor_copy(KV[:], KVN[:])
                # kv columns (fp32 + bf16) for next level
                for mt in range(2):
                    pp = pt.tile([128, 128], f32, tag="pst")
                    nc.tensor.transpose(pp[:, 0:16],
                                        KVN[0:16, mt * 128:(mt + 1) * 128],
                                        IDT[0:16, 0:16])
                    nc.scalar.copy(KVCA[:, mt * G:(mt + 1) * G], pp[:, 0:16])
                    nc.vector.tensor_copy(KVT[mt][:], pp[:, 0:16])
                # cs = kd*kv_new*tanh(s), am = (kv_new-1)*AMV as columns
                nc.vector.tensor_mul(CSC[:], KDCA[:], KVCA[:])
                nc.vector.tensor_mul(CSC[:], CSC[:], THC[:])
                nc.vector.tensor_scalar(AMC[:], KVCA[:], 1.0, AMV,
                                        op0=OP.subtract, op1=OP.mult)
                for mt in range(2):
                    trow(CSB16, CSC[:, mt * G:(mt + 1) * G], mt, out_bf=True)
                    trow(AM3B16, AMC[:, mt * G:(mt + 1) * G], mt, out_bf=True)
                # selector broadcast rows -> psum -> SBUF bf16
                for g in range(G):
                    cb = pbc.tile([128, N], f32, tag="psB")
                    ab = pbc.tile([128, N], f32, tag="psB")
                    nc.tensor.matmul(cb[:], ESEL[:, g * 128:(g + 1) * 128],
                                     CSB16[:], start=True, stop=True)
                    nc.tensor.matmul(ab[:], ESEL[:, g * 128:(g + 1) * 128],
                                     AM3B16[:], start=True, stop=True)
                    sl = slice(g * N, (g + 1) * N)
                    nc.scalar.copy(CSSB[:, sl], cb[:])
                    if g % 2 == 0:
                        nc.scalar.copy(AMSB[:, sl], ab[:])
                    else:
                        nc.vector.tensor_copy(AMSB[:, sl], ab[:])
                # pooled h'' and readout inputs (all-SBUF bf16, on GpSimd)
                for g in range(G):
                    sl = slice(g * N, (g + 1) * N)
                    for ft in range(2):
                        nc.gpsimd.tensor_mul(HM[ft][:, sl], HRAW[ft][:, sl],
                                             CSSB[:, sl])
                        nc.gpsimd.tensor_add(HS[ft][:, sl], HM[ft][:, sl],
                                             AMSB[:, sl])
                # grouped readout reduces
                for ft in range(2):
                    nc.vector.tensor_reduce(
                        RDT[lvl][1][ft][:],
                        HM[ft][:].rearrange("p (g n) -> p g n", g=G),
                        axis=AX.X, op=OP.add)
                    nc.vector.tensor_reduce(
                        RDT[lvl][0][ft][:],
                        HS[ft][:].rearrange("p (g n) -> p g n", g=G),
                        axis=AX.X, op=OP.max)

            # ---- the network
            deg_kd()
            for lvl in range(3):
                dense(lvl + 1)
                prop()
                pool_readout(lvl)
                if lvl < 2:
                    deg_kd()

            # ---- combine readouts: z = sum_lvl [max | mean/k]
            for kind in range(2):
                for ft in range(2):
                    cg = (kind * 2 + ft) * G
                    dst = ZACC[:, cg:cg + G]
                    if kind == 0:
                        nc.vector.tensor_add(dst, RDT[0][0][ft][:],
                                             RDT[1][0][ft][:])
                        nc.vector.tensor_add(dst, dst, RDT[2][0][ft][:])
                    else:
                        nc.vector.tensor_scalar_mul(dst, RDT[0][1][ft][:],
                                                    1.0 / KS[0])
                        for l2 in range(1, 3):
                            nc.vector.tensor_scalar_mul(SCRC[:],
                                                        RDT[l2][1][ft][:],
                                                        1.0 / KS[l2])
                            nc.vector.tensor_add(dst, dst, SCRC[:])

            # ---- final MLP + log_softmax
            ZB = sm.tile([128, 64], bf16, tag="zb")
            nc.vector.tensor_copy(ZB[:], ZACC[:])
            ps1 = pv.tile([16, H1], f32, tag="pcol")
            for kt in range(4):
                nc.tensor.matmul(ps1[0:16, :], ZB[:, kt * 16:(kt + 1) * 16],
                                 W1S[:, kt * H1:(kt + 1) * H1],
                                 start=(kt == 0), stop=False)
            nc.tensor.matmul(ps1[0:16, :], ONEB[0:1, 0:16], B1R[:],
                             start=False, stop=True)
            nc.scalar.activation(Z1[:], ps1[0:16, :], AF.Relu)
            for kt in range(2):
                pp = pt.tile([128, 128], f32, tag="pst")
                nc.tensor.transpose(pp[:, 0:16],
                                    Z1[0:16, kt * 128:(kt + 1) * 128],
                                    IDT[0:16, 0:16])
                nc.scalar.copy(Z1T[:, kt * G:(kt + 1) * G], pp[:, 0:16])
            ps2 = pv.tile([16, H2], f32, tag="pcol")
            for kt in range(2):
                nc.tensor.matmul(ps2[0:16, :], Z1T[:, kt * G:(kt + 1) * G],
                                 W2S[:, kt * H2:(kt + 1) * H2],
                                 start=(kt == 0), stop=False)
            nc.tensor.matmul(ps2[0:16, :], ONEB[0:1, 0:16], B2R[:],
                             start=False, stop=True)
            nc.scalar.activation(Z2[:], ps2[0:16, :], AF.Relu)
            pp = pt.tile([128, 128], f32, tag="pst")
            nc.tensor.transpose(pp[:, 0:16], Z2[0:16, :], IDT[0:16, 0:16])
            nc.scalar.copy(Z2T[:], pp[:, 0:16])
            ps3 = pv.tile([16, C], f32, tag="pcol")
            nc.tensor.matmul(ps3[0:16, :], Z2T[:], W3S[:], start=True,
                             stop=False)
            nc.tensor.matmul(ps3[0:16, :], ONEB[0:1, 0:16], B3R[:],
                             start=False, stop=True)
            nc.vector.tensor_reduce(M16[:], ps3[0:16, :], axis=AX.X, op=OP.max)
            nc.vector.tensor_scalar_mul(NM16[:], M16[:], -1.0)
            nc.scalar.activation(ES[:], ps3[0:16, :], AF.Exp,
                                 bias=NM16[0:16, 0:1], scale=1.0)
            nc.vector.tensor_reduce(SE[:], ES[:], axis=AX.X, op=OP.add)
            nc.scalar.activation(LSE[:], SE[:], AF.Ln)
            nc.vector.tensor_scalar(OUTS[:], ps3[0:16, :], M16[0:16, 0:1],
                                    LSE[0:16, 0:1], op0=OP.subtract,
                                    op1=OP.subtract)
            nc.sync.dma_start(out_d[:], OUTS[:])

    nc.compile()
    return nc


def _get_nc(with_bias):
    key = f"nc{int(with_bias)}"
    if key not in _CACHE:
        _CACHE[key] = _build(with_bias)
    return _CACHE[key]


def _host_prep(inputs):
    import ml_dtypes
    bfd = ml_dtypes.bfloat16
    x = np.asarray(inputs["x"], np.float32)
    edges = np.asarray(inputs["edges"], np.int32)
    src = edges[..., 0].astype(np.int64)
    dst = edges[..., 1].astype(np.int64)
    gidx = np.arange(B, dtype=np.int64)[:, None]
    flat = (gidx * N * N + src * N + dst).ravel()
    A = np.bincount(flat, minlength=B * N * N).astype(np.float32)
    A = A.reshape(B, N, N)
    A += np.eye(N, dtype=np.float32)[None]
    A = A.astype(bfd)
    xt = np.ascontiguousarray(
        x.reshape(NCORES, G * N, F_IN).transpose(0, 2, 1)).astype(bfd)

    esel = np.zeros((G, G * 128), np.float32)
    for g in range(G):
        esel[g, g * 128:(g + 1) * 128] = 1.0

    shared = {"esel": esel.astype(bfd)}
    for name, key in (("w12", "W12"), ("w22", "W22"), ("w32", "W32"),
                      ("w1", "W1"), ("w2", "W2"), ("w3", "W3")):
        shared[name] = np.ascontiguousarray(
            np.asarray(inputs[key], np.float32).astype(bfd))
    for name, key, n in (("b12", "b12", H1), ("b22", "b22", H1),
                         ("b32", "b32", H1), ("b1", "b1", H1),
                         ("b2", "b2", H2), ("b3", "b3", C)):
        shared[name] = np.asarray(inputs[key], np.float32).reshape(1, n) \
            .astype(bfd)
    pwc = np.zeros((128, 6), np.float32)
    for i, key in enumerate(("pw1", "pw2", "pw3")):
        pw = np.asarray(inputs[key], np.float32)
        pwn = pw / np.linalg.norm(pw)
        pwc[:, 2 * i] = pwn[:128]
        pwc[:, 2 * i + 1] = pwn[128:]
    shared["pwc"] = pwc.astype(bfd)

    with_bias = any(np.any(np.asarray(inputs[k])) for k in
                    ("b12", "b22", "b32"))
    in_maps = []
    for c in range(NCORES):
        m = dict(shared)
        m["xt"] = np.ascontiguousarray(xt[c])
        m["adj"] = np.ascontiguousarray(A[c * G:(c + 1) * G])
        in_maps.append(m)
    return in_maps, with_bias


def kernel(**inputs):
    from concourse.bass_utils import run_bass_kernel_spmd
    in_maps, with_bias = _host_prep(inputs)
    nc = _get_nc(with_bias)
    r = run_bass_kernel_spmd(nc, in_maps, core_ids=list(range(NCORES)))
    out = np.concatenate([r.results[c]["out"] for c in range(NCORES)], axis=0)
    return out.astype(np.float32)


def run_traced(inputs):
    """Like kernel() but with NTFF tracing; returns (out, BassKernelResults)."""
    import sys
    import types
    if "antenv.axon_hooks" not in sys.modules:
        hooks = types.ModuleType("antenv.axon_hooks")
        hooks._hook = None
        hooks.set_axon_ntff_profile_hook = lambda h: setattr(hooks, "_hook", h)
        hooks.get_axon_ntff_profile_hook = lambda: hooks._hook
        sys.modules["antenv.axon_hooks"] = hooks
        from trn_agent_boot.trn_boot import _ntff_profile_via_ctypes
        hooks.set_axon_ntff_profile_hook(
            _ntff_profile_via_ctypes("/opt/axon/libaxon_pjrt.so"))
    from concourse.bass_utils import run_bass_kernel_spmd
    in_maps, with_bias = _host_prep(inputs)
    nc = _get_nc(with_bias)
    r = run_bass_kernel_spmd(nc, in_maps, core_ids=list(range(NCORES)),
                             trace=True)
    out = np.concatenate([r.results[c]["out"] for c in range(NCORES)], axis=0)
    return out.astype(np.float32), r
